# revision 5
# baseline (speedup 1.0000x reference)
"""LSEP loss kernel for Trainium2, data-parallel over 8 NeuronCores.

Math per element i (B=1e6, C=10):
  q[c]  = T[i, bayes[i], c]
  s_neg = sum_c (partial[i,c]==0) * exp(q[c])
  s_pos = sum_c (partial[i,c]==1) * exp(-q[c])
  loss  = mean_i log1p(s_neg * s_pos)

Strategy: the loss is a mean, so elements can be freely permuted. Host-side
we bucket elements by bayes value and give every core a static layout of
10 sections (one per bayes value v), each 128 partitions x 100 slots, padded
with null elements (T=0, partial=1 -> term contributes exactly 0). Row
selection then needs no gather at all: section v reads T columns
[v*10, v*10+10) through a static strided access pattern, so no engine ever
touches the 90 unused T values -- they only flow through DMA, which is the
intended memory-bound term. T and the partial mask are staged as fp8(e3m4)
in one 110-byte row per element (100 T values + 10 mask values), giving
~14.1 MB of HBM traffic per core. Per section: ACT computes exp(+-q) from
the strided fp8 view, DVE forms the masked sums via
s_neg = sum(e+) - sum(e+ * p), s_pos = sum(e- * p), then an epilogue does
log1p and a row-sum. Host sums the per-core [128,1] partials and divides
by the true B.
"""

from contextlib import ExitStack

import numpy as np

import concourse.bacc as bacc
import concourse.mybir as mybir
import concourse.tile as tile
from concourse.bass_utils import run_bass_kernel_spmd

f32 = mybir.dt.float32
bf16 = mybir.dt.bfloat16
f8 = mybir.dt.float8e3
Alu = mybir.AluOpType
Act = mybir.ActivationFunctionType
Axis = mybir.AxisListType

B = 1_000_000
C = 10
CC = C * C
ROW = CC + C  # 100 fp8 T values + 10 fp8 mask values
NCORES = 8
V = C  # bayes values / sections
P = 128
NJ = 100  # slots per partition per section
S_V = P * NJ  # 12800 slots per (core, section)
S_CORE = V * S_V  # 128000 slots per core
assert NCORES * S_V >= B // V + 8 * 300  # ~8 sigma headroom per bucket


def build_core_program(nc):
    T_d = nc.dram_tensor("t_in", [S_CORE, ROW], f8, kind="ExternalInput").ap()
    out_d = nc.dram_tensor("sum_out", [P, 1], f32, kind="ExternalOutput").ap()

    view = T_d.rearrange("(v p j) c -> v p (j c)", v=V, p=P, j=NJ)

    with tile.TileContext(nc) as tc, ExitStack() as ctx:
        big = ctx.enter_context(tc.tile_pool(name="big", bufs=5))
        work = ctx.enter_context(tc.tile_pool(name="work", bufs=3))
        acc = ctx.enter_context(tc.tile_pool(name="acc", bufs=1))

        prodbuf = acc.tile([P, V * NJ], f32)

        for v in range(V):
            t = big.tile([P, NJ * ROW], f8, tag="t")
            nc.sync.dma_start(t[:], view[v])
            tv = t[:].rearrange("p (j c) -> p j c", j=NJ)
            tsel = tv[:, :, v * C : (v + 1) * C]  # [128, NJ, 10] T row v
            pm = tv[:, :, CC : CC + C]  # [128, NJ, 10] partial mask

            ep = work.tile([P, NJ * C], bf16, tag="ep")
            epv = ep[:].rearrange("p (j c) -> p j c", j=NJ)
            nc.scalar.activation(epv, tsel, Act.Exp, scale=1.0)
            en = work.tile([P, NJ * C], bf16, tag="en")
            env = en[:].rearrange("p (j c) -> p j c", j=NJ)
            nc.scalar.activation(env, tsel, Act.Exp, scale=-1.0)

            # masked products p*e+ and p*e- on gpsimd (otherwise idle);
            # p is exactly 0.0/1.0 so p*e is bit-exact in bf16
            mn = work.tile([P, NJ * C], bf16, tag="mn")
            mnv = mn[:].rearrange("p (j c) -> p j c", j=NJ)
            nc.gpsimd.tensor_tensor(mnv, epv, pm, op=Alu.mult)
            mp = work.tile([P, NJ * C], bf16, tag="mp")
            mpv = mp[:].rearrange("p (j c) -> p j c", j=NJ)
            nc.gpsimd.tensor_tensor(mpv, env, pm, op=Alu.mult)

            # (1-p)*e+ = e+ - p*e+, bf16 x bf16 -> f32 runs in DVE fast mode
            mn0 = work.tile([P, NJ * C], f32, tag="mn0")
            mn0v = mn0[:].rearrange("p (j c) -> p j c", j=NJ)
            nc.vector.tensor_tensor(mn0v, epv, mnv, op=Alu.subtract)

            s_neg = work.tile([P, NJ], f32, tag="sneg")
            nc.vector.tensor_reduce(s_neg[:], mn0v, axis=Axis.X, op=Alu.add)
            s_pos = work.tile([P, NJ], f32, tag="spos")
            nc.vector.tensor_reduce(s_pos[:], mpv, axis=Axis.X, op=Alu.add)

            nc.vector.tensor_tensor(
                prodbuf[:, v * NJ : (v + 1) * NJ], s_neg[:], s_pos[:], op=Alu.mult
            )

        termbuf = acc.tile([P, V * NJ], f32)
        nc.scalar.activation(termbuf[:], prodbuf[:], Act.Ln, bias=1.0, scale=1.0)
        colsum = acc.tile([P, 1], f32)
        nc.vector.tensor_reduce(colsum[:], termbuf[:], axis=Axis.X, op=Alu.add)
        nc.sync.dma_start(out_d, colsum[:])

    nc.compile()
    return nc


_PROGRAM_CACHE = {}


def _get_program():
    key = (V, NJ)
    if key not in _PROGRAM_CACHE:
        nc = bacc.Bacc("TRN2", target_bir_lowering=False, debug=False)
        build_core_program(nc)
        _PROGRAM_CACHE[key] = nc
    return _PROGRAM_CACHE[key]


def kernel(T, bayes, partial, _trace=False):
    assert T.shape == (B, C, C) and bayes.shape == (B,) and partial.shape == (B, C)
    import ml_dtypes

    f8np = ml_dtypes.float8_e3m4

    # one 110-byte fp8 row per element: [T[i] flattened r-major, partial[i]];
    # row index B is the null element (T=0, partial=1 -> exact 0 contribution)
    R = np.empty((B + 1, ROW), f8np)
    R[:B, :CC] = np.asarray(T, np.float32).reshape(B, CC).astype(f8np)
    R[:B, CC:] = np.asarray(partial).astype(np.float32).astype(f8np)
    R[B, :CC] = 0.0
    R[B, CC:] = 1.0

    bay = np.asarray(bayes).astype(np.int64)
    order = np.argsort(bay, kind="stable")
    counts = np.bincount(bay, minlength=V)
    assert len(counts) == V

    perms = np.full((NCORES, S_CORE), B, dtype=np.int64)
    start = 0
    for v in range(V):
        bucket = order[start : start + counts[v]]
        start += counts[v]
        for k in range(NCORES):
            sub = bucket[k::NCORES]
            assert len(sub) <= S_V, f"bucket overflow v={v} core={k}: {len(sub)}"
            perms[k, v * S_V : v * S_V + len(sub)] = sub

    in_maps = [{"t_in": R[perms[k]]} for k in range(NCORES)]

    nc = _get_program()
    res = run_bass_kernel_spmd(
        nc, in_maps, core_ids=list(range(NCORES)), trace=_trace
    )
    total = sum(
        float(res.results[k]["sum_out"].astype(np.float64).sum())
        for k in range(NCORES)
    )
    out = np.float32(total / B)
    if _trace:
        return out, res
    return out


# revision 9
# speedup vs baseline: 1.1483x; 1.1483x over previous
"""LSEP loss kernel for Trainium2, data-parallel over 8 NeuronCores.

Math per element i (B=1e6, C=10):
  q[c]  = T[i, bayes[i], c]
  s_neg = sum_c (partial[i,c]==0) * exp(q[c])
  s_pos = sum_c (partial[i,c]==1) * exp(-q[c])
  loss  = mean_i log1p(s_neg * s_pos)

Strategy: the loss is a mean, so elements can be freely permuted. Host-side
we bucket elements by bayes value and give every core a static layout of
10 sections (one per bayes value v), each 128 partitions x 100 slots, padded
with null elements (T=0, partial=1 -> term contributes exactly 0). Row
selection then needs no gather: section v reads T columns [v*10, v*10+10)
through a static strided access pattern, so no compute engine touches the 90
unused T values -- they only flow through DMA, which is the intended
memory-bound term. T is staged as fp8(e3m4) rows of 100 bytes; the partial
mask is a separate fp8 tensor loaded once (dense in SBUF, so the gpsimd
mask-multiplies run flat at full rate). Per section: ACT computes exp(+-q)
from the strided fp8 view into dense bf16; gpsimd forms p*e+ and p*e-;
DVE computes (e+ - p*e+) in fast bf16->f32 mode, two innermost reduces and
the s_neg*s_pos product. Epilogue: log1p, row-sum, one [128,1] output per
core; host sums and divides by the true B. All elementwise ops use flat 2D
access patterns -- 3D/strided APs measurably fall off the DVE fast paths.
"""

from contextlib import ExitStack

import numpy as np

import concourse.bacc as bacc
import concourse.mybir as mybir
import concourse.tile as tile
from concourse.bass_utils import run_bass_kernel_spmd

f32 = mybir.dt.float32
bf16 = mybir.dt.bfloat16
f8 = mybir.dt.float8e3
Alu = mybir.AluOpType
Act = mybir.ActivationFunctionType
Axis = mybir.AxisListType

B = 1_000_000
C = 10
CC = C * C
NCORES = 8
V = C  # bayes values / sections
P = 128
NJ = 100  # slots per partition per section
S_V = P * NJ  # 12800 slots per (core, section)
S_CORE = V * S_V  # 128000 slots per core
assert NCORES * S_V >= B // V + 8 * 300  # ~8 sigma headroom per bucket


def build_core_program(nc):
    T_d = nc.dram_tensor("t_in", [S_CORE, CC], f8, kind="ExternalInput").ap()
    m_d = nc.dram_tensor("m_in", [S_CORE, C], f8, kind="ExternalInput").ap()
    out_d = nc.dram_tensor("sum_out", [P, 1], f32, kind="ExternalOutput").ap()

    T_view = T_d.rearrange("(v p j) c -> v p (j c)", v=V, p=P, j=NJ)
    m_view = m_d.rearrange("(v p j) c -> p v (j c)", v=V, p=P, j=NJ)  # [P, V, NJ*C]

    # halves for the final section to shorten the serial tail chain
    subtiles = [(v, 0, NJ) for v in range(V - 1)]
    subtiles += [(V - 1, 0, NJ // 2), (V - 1, NJ // 2, NJ // 2)]

    with tile.TileContext(nc) as tc, ExitStack() as ctx:
        big = ctx.enter_context(tc.tile_pool(name="big", bufs=4))
        work = ctx.enter_context(tc.tile_pool(name="work", bufs=3))
        acc = ctx.enter_context(tc.tile_pool(name="acc", bufs=1))

        prodbuf = acc.tile([P, V * NJ], f32)
        maskbuf = acc.tile([P, V * NJ * C], f8)

        first = True
        for v, j0, nj in subtiles:
            t = big.tile([P, nj * CC], f8, tag="t")
            nc.sync.dma_start(t[:], T_view[v, :, j0 * CC : (j0 + nj) * CC])
            if first:
                # mask for all sections arrives right after the first T tile
                nc.sync.dma_start(
                    maskbuf[:].rearrange("p (v jc) -> p v jc", v=V), m_view
                )
                first = False
            tv = t[:].rearrange("p (j c) -> p j c", j=nj)
            tsel = tv[:, :, v * C : (v + 1) * C]  # [P, nj, 10]: T row v
            pm = maskbuf[:, (v * NJ + j0) * C : (v * NJ + j0 + nj) * C]

            n = nj * C
            ep = work.tile([P, n], bf16, tag="ep")
            nc.scalar.activation(
                ep[:].rearrange("p (j c) -> p j c", j=nj), tsel, Act.Exp, scale=1.0
            )
            en = work.tile([P, n], bf16, tag="en")
            nc.scalar.activation(
                en[:].rearrange("p (j c) -> p j c", j=nj), tsel, Act.Exp, scale=-1.0
            )

            # masked products p*e+ / p*e- on gpsimd (flat, dense operands);
            # p is exactly 0.0/1.0 so p*e is bit-exact in bf16
            mn = work.tile([P, n], bf16, tag="mn")
            nc.gpsimd.tensor_tensor(mn[:], ep[:], pm, op=Alu.mult)
            mp = work.tile([P, n], bf16, tag="mp")
            nc.gpsimd.tensor_tensor(mp[:], en[:], pm, op=Alu.mult)

            # (1-p)*e+ = e+ - p*e+ : flat bf16 x bf16 -> f32 (DVE fast mode)
            mn0 = work.tile([P, n], f32, tag="mn0")
            nc.vector.tensor_tensor(mn0[:], ep[:], mn[:], op=Alu.subtract)

            s_neg = work.tile([P, nj], f32, tag="sneg")
            nc.vector.tensor_reduce(
                s_neg[:], mn0[:].rearrange("p (j c) -> p j c", j=nj),
                axis=Axis.X, op=Alu.add,
            )
            s_pos = work.tile([P, nj], f32, tag="spos")
            nc.vector.tensor_reduce(
                s_pos[:], mp[:].rearrange("p (j c) -> p j c", j=nj),
                axis=Axis.X, op=Alu.add,
            )

            nc.vector.tensor_tensor(
                prodbuf[:, v * NJ + j0 : v * NJ + j0 + nj],
                s_neg[:], s_pos[:], op=Alu.mult,
            )

        termbuf = acc.tile([P, V * NJ], f32)
        nc.scalar.activation(termbuf[:], prodbuf[:], Act.Ln, bias=1.0, scale=1.0)
        colsum = acc.tile([P, 1], f32)
        nc.vector.tensor_reduce(
            colsum[:], termbuf[:].unsqueeze(1), axis=Axis.X, op=Alu.add
        )
        nc.sync.dma_start(out_d, colsum[:])

    nc.compile()
    return nc


_PROGRAM_CACHE = {}


def _get_program():
    key = (V, NJ)
    if key not in _PROGRAM_CACHE:
        nc = bacc.Bacc("TRN2", target_bir_lowering=False, debug=False)
        build_core_program(nc)
        _PROGRAM_CACHE[key] = nc
    return _PROGRAM_CACHE[key]


def kernel(T, bayes, partial, _trace=False):
    assert T.shape == (B, C, C) and bayes.shape == (B,) and partial.shape == (B, C)
    import ml_dtypes

    f8np = ml_dtypes.float8_e3m4

    # fp8 rows: T[i] flattened r-major; null element B: T=0, partial=1
    # (null contributes exactly 0: s_neg = e+ - 1*e+ = 0)
    T8 = np.empty((B + 1, CC), f8np)
    T8[:B] = np.asarray(T, np.float32).reshape(B, CC).astype(f8np)
    T8[B] = 0.0
    p8 = np.empty((B + 1, C), f8np)
    p8[:B] = np.asarray(partial).astype(np.float32).astype(f8np)
    p8[B] = 1.0

    bay = np.asarray(bayes).astype(np.int64)
    order = np.argsort(bay, kind="stable")
    counts = np.bincount(bay, minlength=V)
    assert len(counts) == V

    perms = np.full((NCORES, S_CORE), B, dtype=np.int64)
    start = 0
    for v in range(V):
        bucket = order[start : start + counts[v]]
        start += counts[v]
        for k in range(NCORES):
            sub = bucket[k::NCORES]
            assert len(sub) <= S_V, f"bucket overflow v={v} core={k}: {len(sub)}"
            perms[k, v * S_V : v * S_V + len(sub)] = sub

    in_maps = [{"t_in": T8[perms[k]], "m_in": p8[perms[k]]} for k in range(NCORES)]

    nc = _get_program()
    res = run_bass_kernel_spmd(
        nc, in_maps, core_ids=list(range(NCORES)), trace=_trace
    )
    total = sum(
        float(res.results[k]["sum_out"].astype(np.float64).sum())
        for k in range(NCORES)
    )
    out = np.float32(total / B)
    if _trace:
        return out, res
    return out


# revision 10
# speedup vs baseline: 1.3594x; 1.1838x over previous
"""LSEP loss kernel for Trainium2, data-parallel over 8 NeuronCores.

Math per element i (B=1e6, C=10):
  q[c]  = T[i, bayes[i], c]
  s_neg = sum_c (partial[i,c]==0) * exp(q[c])
  s_pos = sum_c (partial[i,c]==1) * exp(-q[c])
  loss  = mean_i log1p(s_neg * s_pos)

Strategy: the loss is a mean, so elements can be freely permuted. Host-side
we bucket elements by bayes value and give every core a static layout of
10 sections (one per bayes value v), each 128 partitions x 100 slots, padded
with null elements (T=0, partial=1 -> term contributes exactly 0). Row
selection then needs no gather: section v reads T columns [v*10, v*10+10)
through a static strided access pattern, so no compute engine touches the 90
unused T values -- they only flow through DMA, which is the intended
memory-bound term. T is staged as fp8(e3m4) rows of 100 bytes. Both masks
(p and 1-p) are staged as separate fp8 tensors, resident in SBUF after one
DMA each, so each section is exactly: 2 ACT exps (strided fp8 -> dense bf16),
2 flat gpsimd mask-multiplies, 2 DVE innermost-reduces and a small product.
No DVE two-port ops (they lock gpsimd out of SBUF) and minimal SBUF traffic:
under concurrent DMA streaming every engine's effective rate drops ~2x from
port contention, so the op mix matters more than isolated op speed.
Epilogue: log1p, row-sum, one [128,1] output per core; host sums across
cores/partitions and divides by the true B.
"""

from contextlib import ExitStack

import numpy as np

import concourse.bacc as bacc
import concourse.mybir as mybir
import concourse.tile as tile
from concourse.bass_utils import run_bass_kernel_spmd

f32 = mybir.dt.float32
bf16 = mybir.dt.bfloat16
f8 = mybir.dt.float8e3
Alu = mybir.AluOpType
Act = mybir.ActivationFunctionType
Axis = mybir.AxisListType

B = 1_000_000
C = 10
CC = C * C
NCORES = 8
V = C  # bayes values / sections
P = 128
NJ = 100  # slots per partition per section
S_V = P * NJ  # 12800 slots per (core, section)
S_CORE = V * S_V  # 128000 slots per core
assert NCORES * S_V >= B // V + 8 * 300  # ~8 sigma headroom per bucket


def build_core_program(nc):
    T_d = nc.dram_tensor("t_in", [S_CORE, CC], f8, kind="ExternalInput").ap()
    m1_d = nc.dram_tensor("m1_in", [S_CORE, C], f8, kind="ExternalInput").ap()
    m0_d = nc.dram_tensor("m0_in", [S_CORE, C], f8, kind="ExternalInput").ap()
    out_d = nc.dram_tensor("sum_out", [P, 1], f32, kind="ExternalOutput").ap()

    T_view = T_d.rearrange("(v p j) c -> v p (j c)", v=V, p=P, j=NJ)
    m1_view = m1_d.rearrange("(v p j) c -> p v (j c)", v=V, p=P, j=NJ)
    m0_view = m0_d.rearrange("(v p j) c -> p v (j c)", v=V, p=P, j=NJ)

    # halves at both ends: shorter fill ramp and shorter serial tail chain
    subtiles = [(0, 0, NJ // 2), (0, NJ // 2, NJ // 2)]
    subtiles += [(v, 0, NJ) for v in range(1, V - 1)]
    subtiles += [(V - 1, 0, NJ // 2), (V - 1, NJ // 2, NJ // 2)]

    with tile.TileContext(nc) as tc, ExitStack() as ctx:
        big = ctx.enter_context(tc.tile_pool(name="big", bufs=4))
        work = ctx.enter_context(tc.tile_pool(name="work", bufs=3))
        acc = ctx.enter_context(tc.tile_pool(name="acc", bufs=1))

        prodbuf = acc.tile([P, V * NJ], f32)
        mask1 = acc.tile([P, V * NJ * C], f8)
        mask0 = acc.tile([P, V * NJ * C], f8)

        first = True
        for v, j0, nj in subtiles:
            t = big.tile([P, nj * CC], f8, tag="t")
            nc.sync.dma_start(t[:], T_view[v, :, j0 * CC : (j0 + nj) * CC])
            if first:
                # both masks (all sections) arrive right after the first T tile
                nc.sync.dma_start(
                    mask1[:].rearrange("p (v jc) -> p v jc", v=V), m1_view
                )
                nc.sync.dma_start(
                    mask0[:].rearrange("p (v jc) -> p v jc", v=V), m0_view
                )
                first = False
            tv = t[:].rearrange("p (j c) -> p j c", j=nj)
            tsel = tv[:, :, v * C : (v + 1) * C]  # [P, nj, 10]: T row v
            off = (v * NJ + j0) * C
            pm1 = mask1[:, off : off + nj * C]
            pm0 = mask0[:, off : off + nj * C]

            n = nj * C
            ep = work.tile([P, n], bf16, tag="ep")
            nc.scalar.activation(
                ep[:].rearrange("p (j c) -> p j c", j=nj), tsel, Act.Exp, scale=1.0
            )
            en = work.tile([P, n], bf16, tag="en")
            nc.scalar.activation(
                en[:].rearrange("p (j c) -> p j c", j=nj), tsel, Act.Exp, scale=-1.0
            )

            # masked products (1-p)*e+ and p*e- on gpsimd, flat dense aps;
            # masks are exactly 0.0/1.0 so the products are bit-exact in bf16
            mn = work.tile([P, n], bf16, tag="mn")
            nc.gpsimd.tensor_tensor(mn[:], ep[:], pm0, op=Alu.mult)
            mp = work.tile([P, n], bf16, tag="mp")
            nc.gpsimd.tensor_tensor(mp[:], en[:], pm1, op=Alu.mult)

            s_neg = work.tile([P, nj], f32, tag="sneg")
            nc.vector.tensor_reduce(
                s_neg[:], mn[:].rearrange("p (j c) -> p j c", j=nj),
                axis=Axis.X, op=Alu.add,
            )
            s_pos = work.tile([P, nj], f32, tag="spos")
            nc.vector.tensor_reduce(
                s_pos[:], mp[:].rearrange("p (j c) -> p j c", j=nj),
                axis=Axis.X, op=Alu.add,
            )

            nc.vector.tensor_tensor(
                prodbuf[:, v * NJ + j0 : v * NJ + j0 + nj],
                s_neg[:], s_pos[:], op=Alu.mult,
            )

        termbuf = acc.tile([P, V * NJ], f32)
        nc.scalar.activation(termbuf[:], prodbuf[:], Act.Ln, bias=1.0, scale=1.0)
        colsum = acc.tile([P, 1], f32)
        nc.vector.tensor_reduce(
            colsum[:], termbuf[:].unsqueeze(1), axis=Axis.X, op=Alu.add
        )
        nc.sync.dma_start(out_d, colsum[:])

    nc.compile()
    return nc


_PROGRAM_CACHE = {}


def _get_program():
    key = (V, NJ)
    if key not in _PROGRAM_CACHE:
        nc = bacc.Bacc("TRN2", target_bir_lowering=False, debug=False)
        build_core_program(nc)
        _PROGRAM_CACHE[key] = nc
    return _PROGRAM_CACHE[key]


def kernel(T, bayes, partial, _trace=False):
    assert T.shape == (B, C, C) and bayes.shape == (B,) and partial.shape == (B, C)
    import ml_dtypes

    f8np = ml_dtypes.float8_e3m4

    # fp8 rows: T[i] flattened r-major; null element B: T=0, partial=1
    # (null contributes exactly 0: mask0=0 -> s_neg=0 -> log1p(0)=0)
    T8 = np.empty((B + 1, CC), f8np)
    T8[:B] = np.asarray(T, np.float32).reshape(B, CC).astype(f8np)
    T8[B] = 0.0
    p32 = np.asarray(partial).astype(np.float32)
    p81 = np.empty((B + 1, C), f8np)
    p81[:B] = p32
    p81[B] = 1.0
    p80 = np.empty((B + 1, C), f8np)
    p80[:B] = 1.0 - p32
    p80[B] = 0.0

    bay = np.asarray(bayes).astype(np.int64)
    order = np.argsort(bay, kind="stable")
    counts = np.bincount(bay, minlength=V)
    assert len(counts) == V

    perms = np.full((NCORES, S_CORE), B, dtype=np.int64)
    start = 0
    for v in range(V):
        bucket = order[start : start + counts[v]]
        start += counts[v]
        for k in range(NCORES):
            sub = bucket[k::NCORES]
            assert len(sub) <= S_V, f"bucket overflow v={v} core={k}: {len(sub)}"
            perms[k, v * S_V : v * S_V + len(sub)] = sub

    in_maps = [
        {"t_in": T8[perms[k]], "m1_in": p81[perms[k]], "m0_in": p80[perms[k]]}
        for k in range(NCORES)
    ]

    nc = _get_program()
    res = run_bass_kernel_spmd(
        nc, in_maps, core_ids=list(range(NCORES)), trace=_trace
    )
    total = sum(
        float(res.results[k]["sum_out"].astype(np.float64).sum())
        for k in range(NCORES)
    )
    out = np.float32(total / B)
    if _trace:
        return out, res
    return out


# revision 13
# speedup vs baseline: 1.4746x; 1.0847x over previous
"""LSEP loss kernel for Trainium2, data-parallel over 8 NeuronCores.

Math per element i (B=1e6, C=10):
  q[c]  = T[i, bayes[i], c]
  s_neg = sum_c (partial[i,c]==0) * exp(q[c])
  s_pos = sum_c (partial[i,c]==1) * exp(-q[c])
  loss  = mean_i log1p(s_neg * s_pos)

Strategy: the loss is a mean, so elements can be freely permuted. Host-side
we bucket elements by bayes value and give every core a static layout of
10 sections (one per bayes value v), each 128 partitions x 100 slots, padded
with null elements (T=0, partial=1 -> contributes ~2e-7 rel, negligible).
Row selection then needs no gather: section v reads T columns [v*10, v*10+10)
through a static strided access pattern, so no compute engine touches the 90
unused T values -- they only flow through DMA, which is the intended
memory-bound term. T is staged as fp8(e3m4) rows of 100 bytes.

The partial mask is folded ADDITIVELY: the host stages m = -15.5*p as fp8
(both values exact; 15.5 is the e3m4 max normal), and the device computes
u = q + m in one gpsimd add (bf16 out, exact for p=0 since q is fp8).
Then  s_neg = sum_c exp(u)        (masked terms are scaled by e^-15.5)
      s_pos = sum_c exp(-u - 15.5)  (ACT's free scale/bias: func(s*x+b))
so each section is exactly: 1 gpsimd add, 2 ACT exps (flat bf16), 2 DVE
innermost-reduces, 1 small product. No mask multiplies, no DVE two-port ops
(which lock gpsimd out of SBUF), minimal SBUF traffic: under concurrent DMA
streaming every engine's effective rate drops ~2x from port contention, so
the op mix matters more than isolated op speed. Verified numerics vs the
f32 reference: rel err 2.6e-4 (gate 2e-2). Epilogue: log1p, row-sum, one
[128,1] output per core; host sums across cores/partitions, divides by B.
"""

from contextlib import ExitStack

import numpy as np

import concourse.bacc as bacc
import concourse.mybir as mybir
import concourse.tile as tile
from concourse.bass_utils import run_bass_kernel_spmd

f32 = mybir.dt.float32
bf16 = mybir.dt.bfloat16
f8 = mybir.dt.float8e3
Alu = mybir.AluOpType
Act = mybir.ActivationFunctionType
Axis = mybir.AxisListType

B = 1_000_000
C = 10
CC = C * C
NCORES = 8
V = C  # bayes values / sections
P = 128
NJ = 100  # slots per partition per section
S_V = P * NJ  # 12800 slots per (core, section)
S_CORE = V * S_V  # 128000 slots per core
BIG = 15.5  # e3m4 max normal; exp(-BIG) ~ 1.9e-7 suppresses masked terms
assert NCORES * S_V >= B // V + 8 * 300  # ~8 sigma headroom per bucket


def build_core_program(nc):
    T_d = nc.dram_tensor("t_in", [S_CORE, CC], f8, kind="ExternalInput").ap()
    m_d = nc.dram_tensor("m_in", [S_CORE, C], f8, kind="ExternalInput").ap()
    out_d = nc.dram_tensor("sum_out", [P, 1], f32, kind="ExternalOutput").ap()

    T_view = T_d.rearrange("(v p j) c -> v p (j c)", v=V, p=P, j=NJ)
    m_view = m_d.rearrange("(v p j) c -> p v (j c)", v=V, p=P, j=NJ)

    # halves at both ends: shorter fill ramp and shorter serial tail chain
    subtiles = [(0, 0, NJ // 2), (0, NJ // 2, NJ // 2)]
    subtiles += [(v, 0, NJ) for v in range(1, V - 1)]
    subtiles += [(V - 1, 0, NJ // 2), (V - 1, NJ // 2, NJ // 2)]

    with tile.TileContext(nc) as tc, ExitStack() as ctx:
        big = ctx.enter_context(tc.tile_pool(name="big", bufs=4))
        work = ctx.enter_context(tc.tile_pool(name="work", bufs=3))
        acc = ctx.enter_context(tc.tile_pool(name="acc", bufs=1))

        prodbuf = acc.tile([P, V * NJ], f32)
        maskbuf = acc.tile([P, V * NJ * C], f8)
        bigbias = acc.tile([P, 1], f32)
        nc.vector.memset(bigbias[:], -BIG)

        first = True
        for v, j0, nj in subtiles:
            t = big.tile([P, nj * CC], f8, tag="t")
            nc.sync.dma_start(t[:], T_view[v, :, j0 * CC : (j0 + nj) * CC])
            if first:
                # mask for all sections arrives right after the first T tile
                nc.sync.dma_start(
                    maskbuf[:].rearrange("p (v jc) -> p v jc", v=V), m_view
                )
                first = False
            tv = t[:].rearrange("p (j c) -> p j c", j=nj)
            tsel = tv[:, :, v * C : (v + 1) * C]  # [P, nj, 10]: T row v
            off = (v * NJ + j0) * C
            n = nj * C
            pm = maskbuf[:, off : off + n].rearrange("p (j c) -> p j c", j=nj)

            # u = q - BIG*p in one gpsimd add (bf16; exact where p=0)
            u = work.tile([P, n], bf16, tag="u")
            nc.gpsimd.tensor_tensor(
                u[:].rearrange("p (j c) -> p j c", j=nj), tsel, pm, op=Alu.add
            )

            ep = work.tile([P, n], bf16, tag="ep")
            nc.scalar.activation(ep[:], u[:], Act.Exp, scale=1.0)
            en = work.tile([P, n], bf16, tag="en")
            nc.scalar.activation(en[:], u[:], Act.Exp, scale=-1.0, bias=bigbias[:])

            s_neg = work.tile([P, nj], f32, tag="sneg")
            nc.vector.tensor_reduce(
                s_neg[:], ep[:].rearrange("p (j c) -> p j c", j=nj),
                axis=Axis.X, op=Alu.add,
            )
            s_pos = work.tile([P, nj], f32, tag="spos")
            nc.vector.tensor_reduce(
                s_pos[:], en[:].rearrange("p (j c) -> p j c", j=nj),
                axis=Axis.X, op=Alu.add,
            )

            nc.vector.tensor_tensor(
                prodbuf[:, v * NJ + j0 : v * NJ + j0 + nj],
                s_neg[:], s_pos[:], op=Alu.mult,
            )

        termbuf = acc.tile([P, V * NJ], f32)
        nc.scalar.activation(termbuf[:], prodbuf[:], Act.Ln, bias=1.0, scale=1.0)
        colsum = acc.tile([P, 1], f32)
        nc.vector.tensor_reduce(
            colsum[:], termbuf[:].unsqueeze(1), axis=Axis.X, op=Alu.add
        )
        nc.sync.dma_start(out_d, colsum[:])

    nc.compile()
    return nc


_PROGRAM_CACHE = {}


def _get_program():
    key = (V, NJ)
    if key not in _PROGRAM_CACHE:
        nc = bacc.Bacc("TRN2", target_bir_lowering=False, debug=False)
        build_core_program(nc)
        _PROGRAM_CACHE[key] = nc
    return _PROGRAM_CACHE[key]


def kernel(T, bayes, partial, _trace=False):
    assert T.shape == (B, C, C) and bayes.shape == (B,) and partial.shape == (B, C)
    import ml_dtypes

    f8np = ml_dtypes.float8_e3m4

    # fp8 rows: T[i] flattened r-major; null element B: T=0, partial=1
    T8 = np.empty((B + 1, CC), f8np)
    T8[:B] = np.asarray(T, np.float32).reshape(B, CC).astype(f8np)
    T8[B] = 0.0
    m8 = np.empty((B + 1, C), f8np)
    m8[:B] = np.asarray(partial).astype(np.float32) * (-BIG)
    m8[B] = -BIG

    bay = np.asarray(bayes).astype(np.int64)
    order = np.argsort(bay, kind="stable")
    counts = np.bincount(bay, minlength=V)
    assert len(counts) == V

    perms = np.full((NCORES, S_CORE), B, dtype=np.int64)
    start = 0
    for v in range(V):
        bucket = order[start : start + counts[v]]
        start += counts[v]
        for k in range(NCORES):
            sub = bucket[k::NCORES]
            assert len(sub) <= S_V, f"bucket overflow v={v} core={k}: {len(sub)}"
            perms[k, v * S_V : v * S_V + len(sub)] = sub

    in_maps = [{"t_in": T8[perms[k]], "m_in": m8[perms[k]]} for k in range(NCORES)]

    nc = _get_program()
    res = run_bass_kernel_spmd(
        nc, in_maps, core_ids=list(range(NCORES)), trace=_trace
    )
    total = sum(
        float(res.results[k]["sum_out"].astype(np.float64).sum())
        for k in range(NCORES)
    )
    out = np.float32(total / B)
    if _trace:
        return out, res
    return out
